# revision 2
# baseline (speedup 1.0000x reference)
"""GAT head kernel for Trainium2, 8 SPMD NeuronCores (v2).

Reference (B=4, N=4096, D=256):
    feats  = data @ W1.T                          [B,N,D]
    f1     = feats @ W2 + b2                      [B,N]
    logits = f1[:,:,None] + f1[:,None,:]          [B,N,N]
    coefs  = softmax(leaky_relu(logits) + bias1, axis=-1)
    out    = coefs @ feats + bias2 + data

Core c = 2*b + h owns batch b, row half h (R=2048 rows i), needs all N j's.
E[j,i] = exp(leaky_relu(f1_i+f1_j) + bias1[i,j]) is built in [j(part), i]
tiles (the lhsT the PE wants); a ones column in the rhs yields the softmax
denominator in the same matmul; bias2 + data residual are folded into the
host-prepped `datan` so the normalize is one STT.

Per (ic, jb) tile [128, 512] one of two pipelines (knob: Bresenham split):
  A (DVE-heavy, bf16, max-approx of the leaky branch):
     ea = abc * a_j      (TS)    abc = exp(f1bc) per ic, a_j = exp(f1_j+2b2)
     t  = tbc + s_j      (TS)    tbc = 1 + 0.01 f1bc,    s_j = .01 f1_j+.02 b2
     m  = max(ea, t)     (TT)
     e  = m * exp(b1)    (TT)    exp(bias1) bf16 block from HBM
  D (ACT-heavy, exact leaky, fp16 logits):
     lr = Lrelu(f1bc + f1_j + 2b2)    (ACT, alpha=.01, fp16 out)
     z  = lr + bias1_raw(fp16)        (TT)
     e  = exp(z)                      (ACT, bf16 out)
Host uploads, per (ic, jb) block, either exp(bias1) as bf16 bits or raw
bias1 as fp16 bits in ONE uint16 tensor laid out [ic, g, p, q, i] so each
DMA line is 8KB/partition contiguous.
"""

import sys

sys.path.insert(0, "/opt/trn_rl_repo")

import numpy as np
import ml_dtypes

import concourse.bass as bass
import concourse.mybir as mybir
from concourse.tile import TileContext
from concourse.bass_utils import run_bass_kernel_spmd

# ---------------------------------------------------------------- config
B, N, D = 4, 4096, 256
NCORES = 8
R = N * B // NCORES          # rows per core = 2048
NB = N // 128                # j blocks = 32
IC = 512                     # i-chunk width
NIC = R // IC                # i chunks per core = 4
HB = R // 128                # 16: i-blocks of 128 per core

F32 = mybir.dt.float32
BF16 = mybir.dt.bfloat16
F16 = mybir.dt.float16
U16 = mybir.dt.uint16

# knobs
D_CUT = 20                   # of every 32 jb's, this many go down pipeline D
FB_ACT_MOD = 2               # fb copies: jb % FB_ACT_MOD == 0 -> ACT, else DVE

_nc_cache = {}


def is_d_tile(jb):
    """Bresenham-spread D_CUT of NB j-blocks to pipeline D (same fn host+dev)."""
    return ((jb + 1) * D_CUT) // NB > (jb * D_CUT) // NB


def _legalize_waits(nc, max_inst_waits=1, max_ev_waits=2):
    """Hoist excess sync waits into EventSemaphores on the same engine."""
    counter = 0
    for fn in nc.m.functions:
        for bb in fn.blocks:
            out = []
            changed = False
            for ins in bb.instructions:
                si = ins.sync_info
                waits = list(si.on_wait) if si and si.on_wait else []
                limit = (
                    max_ev_waits
                    if isinstance(ins, mybir.InstEventSemaphore)
                    else max_inst_waits
                )
                if len(waits) > limit:
                    extra, keep = waits[:-limit], waits[-limit:]
                    while extra:
                        chunk, extra = extra[:max_ev_waits], extra[max_ev_waits:]
                        counter += 1
                        ev = mybir.InstEventSemaphore(
                            name=f"waitsplit_{counter}", engine=ins.engine
                        )
                        ev.sync_info = mybir.SyncInfo(on_wait=chunk, on_update=[])
                        out.append(ev)
                        changed = True
                    ins.sync_info = mybir.SyncInfo(
                        on_wait=keep,
                        on_update=list(si.on_update) if si.on_update else [],
                    )
                out.append(ins)
            if changed:
                bb.instructions = out
    return nc


def build_nc():
    key = (D_CUT, FB_ACT_MOD, IC)
    if key in _nc_cache:
        return _nc_cache[key]

    nc = bass.Bass()
    AF = mybir.ActivationFunctionType
    OP = mybir.AluOpType

    dataT_d = nc.dram_tensor("dataT", [D, N], BF16, kind="ExternalInput")
    datan_d = nc.dram_tensor("datan", [R, D], F32, kind="ExternalInput")
    w1t_d = nc.dram_tensor("w1t", [D, D + 1], BF16, kind="ExternalInput")
    b2rep_d = nc.dram_tensor("b2rep", [128, 1], F32, kind="ExternalInput")
    b1t_d = nc.dram_tensor("b1t", [NIC, 4, 128, 8, IC], U16, kind="ExternalInput")
    ident_d = nc.dram_tensor("ident", [128, 128], F32, kind="ExternalInput")
    out_d = nc.dram_tensor("out", [R, D], F32, kind="ExternalOutput")

    with TileContext(nc) as tc:
        with (
            tc.tile_pool(name="persist", bufs=1) as pp,
            tc.tile_pool(name="epool", bufs=2) as ep,
            tc.tile_pool(name="stream", bufs=6) as sp,
            tc.tile_pool(name="psum", bufs=3, space="PSUM") as psp,
            tc.tile_pool(name="psfeat", bufs=2, space="PSUM") as psf,
        ):
            # ---------------- phase 0: feats, f1, broadcast rows ----------
            dT0 = pp.tile([128, N], BF16, tag="dT0")
            dT1 = pp.tile([128, N], BF16, tag="dT1")
            HALF = HB * 128
            Q = HALF // 2
            nc.sync.dma_start(dT0[:, 0:Q], dataT_d[0:128, 0:Q])
            nc.sync.dma_start(dT1[:, 0:Q], dataT_d[128:256, 0:Q])

            # w1t host-augmented to [256, 257]: col 256 = W1.T @ W2 so the
            # feats matmul also emits raw f1 in psum col 256.
            w1t_lo = pp.tile([128, D + 1], BF16, tag="w1lo")
            w1t_hi = pp.tile([128, D + 1], BF16, tag="w1hi")
            nc.sync.dma_start(w1t_lo[:], w1t_d[0:128, :])
            nc.sync.dma_start(w1t_hi[:], w1t_d[128:256, :])
            b2rep = pp.tile([128, 1], F32, tag="b2rep")
            nc.sync.dma_start(b2rep[:], b2rep_d[:])
            b2s = pp.tile([128, 1], F32, tag="b2s")
            nc.vector.tensor_scalar_mul(b2s[:], b2rep[:], 0.01)
            nc.sync.dma_start(dT0[:, Q:HALF], dataT_d[0:128, Q:HALF])
            nc.sync.dma_start(dT1[:, Q:HALF], dataT_d[128:256, Q:HALF])
            nc.sync.dma_start(dT0[:, HALF:N], dataT_d[0:128, HALF:N])
            nc.sync.dma_start(dT1[:, HALF:N], dataT_d[128:256, HALF:N])
            ident = pp.tile([128, 128], F32, tag="ident")
            nc.sync.dma_start(ident[:], ident_d[:])
            ones128 = pp.tile([128, 128], F32, tag="ones128")
            nc.vector.memset(ones128[:], 1.0)

            # feats bf16 with ones column at [:, :, 256] for the denominator
            fb = pp.tile([128, NB, D + 2], BF16, tag="fb")
            nc.vector.memset(fb[:, :, D : D + 1], 1.0)
            f1 = pp.tile([128, NB], F32, tag="f1")        # raw f1 (no b2)
            f1b2 = pp.tile([128, NB], F32, tag="f1b2")    # f1 + 2*b2
            s001 = pp.tile([128, NB], F32, tag="s001")    # .01 f1 + .02 b2
            aj = pp.tile([128, NB], F32, tag="aj")        # exp(f1 + 2 b2)
            f1bc = pp.tile([128, R], F32, tag="f1bc")

            for jb in range(NB):
                jsl = slice(jb * 128, (jb + 1) * 128)
                ps = psf.tile([128, D + 1], F32, tag="featps")
                nc.tensor.matmul(ps[:], dT0[:, jsl], w1t_lo[:], start=True, stop=False)
                nc.tensor.matmul(ps[:], dT1[:, jsl], w1t_hi[:], start=False, stop=True)
                if jb % FB_ACT_MOD == 0:
                    nc.scalar.copy(fb[:, jb, 0:D], ps[:, 0:D])
                else:
                    nc.vector.tensor_copy(fb[:, jb, 0:D], ps[:, 0:D])
                nc.vector.tensor_copy(f1[:, jb : jb + 1], ps[:, D : D + 1])
                if jb < HB:
                    # own-row broadcast: replicate f1 col jb along free,
                    # PE-transpose into f1bc block (f1 of row i, all parts).
                    colt = sp.tile([128, 128], F32, tag="colt", bufs=3)
                    nc.vector.tensor_scalar_mul(
                        colt[:], ones128[:], f1[:, jb : jb + 1]
                    )
                    psb = psf.tile([128, 128], F32, tag="psb")
                    nc.tensor.transpose(psb[:], colt[:], ident[:])
                    nc.vector.tensor_copy(
                        f1bc[:, jb * 128 : (jb + 1) * 128], psb[:]
                    )
                if jb == HB - 1 or jb == NB - 1:
                    hsl = slice(0, HB) if jb == HB - 1 else slice(HB, NB)
                    nc.vector.tensor_scalar(
                        f1b2[:, hsl], f1[:, hsl], b2rep[:, 0:1], None, OP.add
                    )
                    nc.vector.tensor_scalar(
                        s001[:, hsl], f1[:, hsl], 0.01, b2s[:, 0:1],
                        OP.mult, OP.add,
                    )
                    nc.scalar.activation(
                        aj[:, hsl], f1b2[:, hsl], AF.Exp, bias=0.0, scale=1.0
                    )

            # ---------------- phase 1: E tiles + matmul ----------
            datan_r = datan_d.rearrange("(rb p) o -> p rb o", p=128)
            out_r = out_d.rearrange("(rb p) o -> p rb o", p=128)
            for ic in range(NIC):
                icsl = slice(ic * IC, (ic + 1) * IC)
                e = ep.tile([128, NB, IC], BF16, tag="e")
                b1g = [None] * 4
                for g in range(4):
                    b1g[g] = sp.tile([128, 8, IC], U16, bufs=2,
                                     name=f"b1g{g}", tag=f"b1g{g % 2}")
                    nc.sync.dma_start(b1g[g][:], b1t_d[ic, g])
                dnb = sp.tile([128, 4, D], F32, tag="dnb", bufs=2)
                nc.sync.dma_start(dnb[:], datan_r[:, ic * 4 : (ic + 1) * 4, :])
                obuf = sp.tile([128, 4, D], F32, tag="obuf", bufs=2)

                # per-ic broadcast tiles for pipeline A
                abc = sp.tile([128, IC], BF16, tag="abc", bufs=2)
                nc.scalar.activation(abc[:], f1bc[:, icsl], AF.Exp,
                                     bias=0.0, scale=1.0)
                tbc = sp.tile([128, IC], BF16, tag="tbc", bufs=2)
                nc.vector.tensor_scalar(tbc[:], f1bc[:, icsl], 0.01, 1.0,
                                        OP.mult, OP.add)

                for jb in range(NB):
                    bslice = b1g[jb // 8][:, jb % 8, :]
                    if is_d_tile(jb):
                        lr = sp.tile([128, IC], F16, tag="lr")
                        nc.scalar.activation(
                            lr[:], f1bc[:, icsl], AF.Lrelu,
                            bias=f1b2[:, jb : jb + 1], scale=1.0, alpha=0.01,
                        )
                        z = sp.tile([128, IC], F16, tag="z")
                        nc.vector.tensor_tensor(
                            z[:], lr[:], bslice.bitcast(F16), OP.add
                        )
                        nc.scalar.activation(
                            e[:, jb, :], z[:], AF.Exp, bias=0.0, scale=1.0
                        )
                    else:
                        ea = sp.tile([128, IC], BF16, tag="ea")
                        nc.vector.tensor_scalar(
                            ea[:], abc[:], aj[:, jb : jb + 1], None, OP.mult
                        )
                        tt = sp.tile([128, IC], BF16, tag="tt")
                        nc.vector.tensor_scalar(
                            tt[:], tbc[:], s001[:, jb : jb + 1], None, OP.add
                        )
                        m = sp.tile([128, IC], BF16, tag="m")
                        nc.vector.tensor_tensor(m[:], ea[:], tt[:], OP.max)
                        nc.vector.tensor_tensor(
                            e[:, jb, :], m[:], bslice.bitcast(BF16), OP.mult
                        )

                for i128 in range(IC // 128):
                    acc = psp.tile([128, D + 1], F32, tag="acc")
                    for jb in range(NB):
                        nc.tensor.matmul(
                            acc[:],
                            e[:, jb, i128 * 128 : (i128 + 1) * 128],
                            fb[:, jb, 0 : D + 1],
                            start=(jb == 0),
                            stop=(jb == NB - 1),
                        )
                    rcp = sp.tile([128, 1], F32, tag="rcp")
                    nc.vector.reciprocal(rcp[:], acc[:, D : D + 1])
                    nc.vector.scalar_tensor_tensor(
                        obuf[:, i128, :], acc[:, 0:D], rcp[:, 0:1],
                        dnb[:, i128, :], OP.mult, OP.add,
                    )
                nc.sync.dma_start(out_r[:, ic * 4 : (ic + 1) * 4, :], obuf[:])

    _legalize_waits(nc)
    _nc_cache[key] = nc
    return nc


def make_in_maps(data, bias1, W1, W2, b2, bias2):
    """Host-side sharding / layout prep. Core c = 2*b + h."""
    data = np.asarray(data, dtype=np.float32)
    bias1 = np.asarray(bias1, dtype=np.float32)
    W1 = np.asarray(W1, dtype=np.float32)
    W2 = np.asarray(W2, dtype=np.float32)
    b2 = np.asarray(b2, dtype=np.float32)
    bias2 = np.asarray(bias2, dtype=np.float32)

    bf = ml_dtypes.bfloat16
    # augmented weights (see build_nc)
    w1t = np.zeros((D, D + 1), dtype=np.float32)
    w1t[:, 0:D] = W1.T
    w1t[:, D] = (W1.astype(np.float64).T @ W2.astype(np.float64)).astype(np.float32)
    w1tb = w1t.astype(bf)
    b2rep = np.full((128, 1), 2.0 * b2[0], dtype=np.float32)
    ident = np.eye(128, dtype=np.float32)

    b1T = bias1.T  # [j, i]
    d_tiles = [jb for jb in range(NB) if is_d_tile(jb)]

    in_maps = []
    for c in range(NCORES):
        b, h = divmod(c, 2)
        rows = slice(h * R, (h + 1) * R)
        dT = data[b].T  # [D, N]
        if h == 1:
            dT = np.concatenate([dT[:, R:], dT[:, :R]], axis=1)
            eb = np.concatenate([b1T[R:, rows], b1T[:R, rows]], axis=0)
        else:
            eb = b1T[:, rows]
        # per-(ic, jb) blocks: raw fp16 bits for D tiles, exp bf16 for A
        b1t = np.empty((NIC, 4, 128, 8, IC), dtype=np.uint16)
        for jb in range(NB):
            g, q = divmod(jb, 8)
            blk = eb[jb * 128 : (jb + 1) * 128, :]  # [128, R]
            if jb in d_tiles:
                bits = blk.astype(np.float16).view(np.uint16)
            else:
                bits = np.exp(blk).astype(bf).view(np.uint16)
            for ic in range(NIC):
                b1t[ic, g, :, q, :] = bits[:, ic * IC : (ic + 1) * IC]
        in_maps.append(
            {
                "dataT": np.ascontiguousarray(dT.astype(bf)),
                "datan": np.ascontiguousarray(data[b, rows] + bias2[None, :]),
                "w1t": w1tb,
                "b2rep": b2rep,
                "b1t": b1t,
                "ident": ident,
            }
        )
    return in_maps


def assemble(results):
    out = np.empty((B, N, D), dtype=np.float32)
    for c in range(NCORES):
        b, h = divmod(c, 2)
        out[b, h * R : (h + 1) * R, :] = results[c]["out"]
    return out


def kernel(data, bias1, W1, W2, b2, bias2):
    nc = build_nc()
    in_maps = make_in_maps(data, bias1, W1, W2, b2, bias2)
    res = run_bass_kernel_spmd(nc, in_maps, core_ids=list(range(NCORES)))
    return assemble(res.results)


# revision 4
# speedup vs baseline: 1.1002x; 1.1002x over previous
"""GAT head kernel for Trainium2, 8 SPMD NeuronCores (v2).

Reference (B=4, N=4096, D=256):
    feats  = data @ W1.T                          [B,N,D]
    f1     = feats @ W2 + b2                      [B,N]
    logits = f1[:,:,None] + f1[:,None,:]          [B,N,N]
    coefs  = softmax(leaky_relu(logits) + bias1, axis=-1)
    out    = coefs @ feats + bias2 + data

Core c = 2*b + h owns batch b, row half h (R=2048 rows i), needs all N j's.
E[j,i] = exp(leaky_relu(f1_i+f1_j) + bias1[i,j]) is built in [j(part), i]
tiles (the lhsT the PE wants); a ones column in the rhs yields the softmax
denominator in the same matmul; bias2 + data residual are folded into the
host-prepped `datan` so the normalize is one STT.

Per (ic, jb) tile [128, 512] one of two pipelines (knob: Bresenham split):
  A (DVE-heavy, bf16, max-approx of the leaky branch):
     ea = abc * a_j      (TS)    abc = exp(f1bc) per ic, a_j = exp(f1_j+2b2)
     t  = tbc + s_j      (TS)    tbc = 1 + 0.01 f1bc,    s_j = .01 f1_j+.02 b2
     m  = max(ea, t)     (TT)
     e  = m * exp(b1)    (TT)    exp(bias1) bf16 block from HBM
  D (ACT-heavy, exact leaky, fp16 logits):
     lr = Lrelu(f1bc + f1_j + 2b2)    (ACT, alpha=.01, fp16 out)
     z  = lr + bias1_raw(fp16)        (TT)
     e  = exp(z)                      (ACT, bf16 out)
Host uploads, per (ic, jb) block, either exp(bias1) as bf16 bits or raw
bias1 as fp16 bits in ONE uint16 tensor laid out [ic, g, p, q, i] so each
DMA line is 8KB/partition contiguous.
"""

import sys

sys.path.insert(0, "/opt/trn_rl_repo")

import numpy as np
import ml_dtypes

import concourse.bass as bass
import concourse.mybir as mybir
from concourse.tile import TileContext
from concourse.bass_utils import run_bass_kernel_spmd

# ---------------------------------------------------------------- config
B, N, D = 4, 4096, 256
NCORES = 8
R = N * B // NCORES          # rows per core = 2048
NB = N // 128                # j blocks = 32
IC = 512                     # i-chunk width
NIC = R // IC                # i chunks per core = 4
HB = R // 128                # 16: i-blocks of 128 per core

F32 = mybir.dt.float32
BF16 = mybir.dt.bfloat16
F16 = mybir.dt.float16
U16 = mybir.dt.uint16

# knobs
D_CUT = 20                   # of every 32 jb's, this many go down pipeline D
FB_ACT_MOD = 2               # fb copies: jb % FB_ACT_MOD == 0 -> ACT, else DVE

_nc_cache = {}


def is_d_tile(jb):
    """Bresenham-spread D_CUT of NB j-blocks to pipeline D (same fn host+dev)."""
    return ((jb + 1) * D_CUT) // NB > (jb * D_CUT) // NB


def _legalize_waits(nc, max_inst_waits=1, max_ev_waits=2):
    """Hoist excess sync waits into EventSemaphores on the same engine."""
    counter = 0
    for fn in nc.m.functions:
        for bb in fn.blocks:
            out = []
            changed = False
            for ins in bb.instructions:
                si = ins.sync_info
                waits = list(si.on_wait) if si and si.on_wait else []
                limit = (
                    max_ev_waits
                    if isinstance(ins, mybir.InstEventSemaphore)
                    else max_inst_waits
                )
                if len(waits) > limit:
                    extra, keep = waits[:-limit], waits[-limit:]
                    while extra:
                        chunk, extra = extra[:max_ev_waits], extra[max_ev_waits:]
                        counter += 1
                        ev = mybir.InstEventSemaphore(
                            name=f"waitsplit_{counter}", engine=ins.engine
                        )
                        ev.sync_info = mybir.SyncInfo(on_wait=chunk, on_update=[])
                        out.append(ev)
                        changed = True
                    ins.sync_info = mybir.SyncInfo(
                        on_wait=keep,
                        on_update=list(si.on_update) if si.on_update else [],
                    )
                out.append(ins)
            if changed:
                bb.instructions = out
    return nc


def build_nc():
    key = (D_CUT, FB_ACT_MOD, IC)
    if key in _nc_cache:
        return _nc_cache[key]

    nc = bass.Bass()
    AF = mybir.ActivationFunctionType
    OP = mybir.AluOpType

    dataT_d = nc.dram_tensor("dataT", [D, N], BF16, kind="ExternalInput")
    datan_d = nc.dram_tensor("datan", [R, D], F32, kind="ExternalInput")
    w1t_d = nc.dram_tensor("w1t", [D, D + 1], BF16, kind="ExternalInput")
    b2rep_d = nc.dram_tensor("b2rep", [128, 1], F32, kind="ExternalInput")
    b1t_d = nc.dram_tensor("b1t", [NIC, 4, 128, 8, IC], U16, kind="ExternalInput")
    ident_d = nc.dram_tensor("ident", [128, 128], F32, kind="ExternalInput")
    out_d = nc.dram_tensor("out", [R, D], F32, kind="ExternalOutput")

    with TileContext(nc) as tc:
        with (
            tc.tile_pool(name="persist", bufs=1) as pp,
            tc.tile_pool(name="epool", bufs=2) as ep,
            tc.tile_pool(name="stream", bufs=6) as sp,
            tc.tile_pool(name="psum", bufs=3, space="PSUM") as psp,
            tc.tile_pool(name="psfeat", bufs=2, space="PSUM") as psf,
        ):
            # ---------------- phase 0: feats, f1, broadcast rows ----------
            dT0 = pp.tile([128, N], BF16, tag="dT0")
            dT1 = pp.tile([128, N], BF16, tag="dT1")
            HALF = HB * 128
            Q = HALF // 2
            nc.sync.dma_start(dT0[:, 0:Q], dataT_d[0:128, 0:Q])
            nc.sync.dma_start(dT1[:, 0:Q], dataT_d[128:256, 0:Q])

            # w1t host-augmented to [256, 257]: col 256 = W1.T @ W2 so the
            # feats matmul also emits raw f1 in psum col 256.
            w1t_lo = pp.tile([128, D + 1], BF16, tag="w1lo")
            w1t_hi = pp.tile([128, D + 1], BF16, tag="w1hi")
            nc.sync.dma_start(w1t_lo[:], w1t_d[0:128, :])
            nc.sync.dma_start(w1t_hi[:], w1t_d[128:256, :])
            b2rep = pp.tile([128, 1], F32, tag="b2rep")
            nc.sync.dma_start(b2rep[:], b2rep_d[:])
            b2s = pp.tile([128, 1], F32, tag="b2s")
            nc.vector.tensor_scalar_mul(b2s[:], b2rep[:], 0.01)
            nc.sync.dma_start(dT0[:, Q:HALF], dataT_d[0:128, Q:HALF])
            nc.sync.dma_start(dT1[:, Q:HALF], dataT_d[128:256, Q:HALF])
            nc.sync.dma_start(dT0[:, HALF:N], dataT_d[0:128, HALF:N])
            nc.sync.dma_start(dT1[:, HALF:N], dataT_d[128:256, HALF:N])
            ident = pp.tile([128, 128], F32, tag="ident")
            nc.sync.dma_start(ident[:], ident_d[:])
            ones128 = pp.tile([128, 128], F32, tag="ones128")
            nc.vector.memset(ones128[:], 1.0)

            # feats bf16 with ones column at [:, :, 256] for the denominator
            fb = pp.tile([128, NB, D + 2], BF16, tag="fb")
            nc.vector.memset(fb[:, :, D : D + 1], 1.0)
            f1 = pp.tile([128, NB], F32, tag="f1")        # raw f1 (no b2)
            f1b2 = pp.tile([128, NB], F32, tag="f1b2")    # f1 + 2*b2
            s001 = pp.tile([128, NB], F32, tag="s001")    # .01 f1 + .02 b2
            aj = pp.tile([128, NB], F32, tag="aj")        # exp(f1 + 2 b2)
            f1bc = pp.tile([128, R], F32, tag="f1bc")

            for jb in range(NB):
                jsl = slice(jb * 128, (jb + 1) * 128)
                ps = psf.tile([128, D + 1], F32, tag="featps")
                nc.tensor.matmul(ps[:], dT0[:, jsl], w1t_lo[:], start=True, stop=False)
                nc.tensor.matmul(ps[:], dT1[:, jsl], w1t_hi[:], start=False, stop=True)
                nc.vector.tensor_copy(fb[:, jb, 0:D], ps[:, 0:D])
                nc.vector.tensor_copy(f1[:, jb : jb + 1], ps[:, D : D + 1])
                if jb < HB:
                    # own-row broadcast: replicate f1 col jb along free,
                    # PE-transpose into f1bc block (f1 of row i, all parts).
                    colt = sp.tile([128, 128], F32, tag="colt", bufs=3)
                    nc.vector.tensor_scalar_mul(
                        colt[:], ones128[:], f1[:, jb : jb + 1]
                    )
                    psb = psf.tile([128, 128], F32, tag="psb")
                    nc.tensor.transpose(psb[:], colt[:], ident[:])
                    nc.vector.tensor_copy(
                        f1bc[:, jb * 128 : (jb + 1) * 128], psb[:]
                    )
                if jb == HB - 1 or jb == NB - 1:
                    hsl = slice(0, HB) if jb == HB - 1 else slice(HB, NB)
                    nc.vector.tensor_scalar(
                        f1b2[:, hsl], f1[:, hsl], b2rep[:, 0:1], None, OP.add
                    )
                    nc.vector.tensor_scalar(
                        s001[:, hsl], f1[:, hsl], 0.01, b2s[:, 0:1],
                        OP.mult, OP.add,
                    )
                    nc.scalar.activation(
                        aj[:, hsl], f1b2[:, hsl], AF.Exp, bias=0.0, scale=1.0
                    )

            # ---------------- phase 1: E tiles + matmul ----------
            datan_r = datan_d.rearrange("(rb p) o -> p rb o", p=128)
            out_r = out_d.rearrange("(rb p) o -> p rb o", p=128)
            for ic in range(NIC):
                icsl = slice(ic * IC, (ic + 1) * IC)
                e = ep.tile([128, NB, IC], BF16, tag="e")
                b1g = [None] * 4
                for g in range(4):
                    b1g[g] = sp.tile([128, 8, IC], U16, bufs=2,
                                     name=f"b1g{g}", tag=f"b1g{g % 2}")
                    nc.sync.dma_start(b1g[g][:], b1t_d[ic, g])
                dnb = sp.tile([128, 4, D], F32, tag="dnb", bufs=2)
                nc.sync.dma_start(dnb[:], datan_r[:, ic * 4 : (ic + 1) * 4, :])
                obuf = sp.tile([128, 4, D], F32, tag="obuf", bufs=2)

                # per-ic broadcast tiles for pipeline A
                abc = sp.tile([128, IC], BF16, tag="abc", bufs=2)
                nc.scalar.activation(abc[:], f1bc[:, icsl], AF.Exp,
                                     bias=0.0, scale=1.0)
                tbc = sp.tile([128, IC], BF16, tag="tbc", bufs=2)
                nc.vector.tensor_scalar(tbc[:], f1bc[:, icsl], 0.01, 1.0,
                                        OP.mult, OP.add)

                # D pipeline in 3 batched stages so the ACT table only
                # reloads twice per ic (Lrelu batch, then Exp batch),
                # not on every Lrelu<->Exp switch.
                d_jbs = [jb for jb in range(NB) if is_d_tile(jb)]
                lrt = sp.tile([128, len(d_jbs), IC], F16, tag="lrt", bufs=1)
                for jd, jb in enumerate(d_jbs):
                    nc.scalar.activation(
                        lrt[:, jd, :], f1bc[:, icsl], AF.Lrelu,
                        bias=f1b2[:, jb : jb + 1], scale=1.0, alpha=0.01,
                    )
                for jb in range(NB):
                    bslice = b1g[jb // 8][:, jb % 8, :]
                    if is_d_tile(jb):
                        jd = d_jbs.index(jb)
                        nc.vector.tensor_tensor(
                            lrt[:, jd, :], lrt[:, jd, :],
                            bslice.bitcast(F16), OP.add,
                        )
                    else:
                        ea = sp.tile([128, IC], BF16, tag="ea", bufs=3)
                        nc.vector.tensor_scalar(
                            ea[:], abc[:], aj[:, jb : jb + 1], None, OP.mult
                        )
                        tt = sp.tile([128, IC], BF16, tag="tt", bufs=3)
                        nc.vector.tensor_scalar(
                            tt[:], tbc[:], s001[:, jb : jb + 1], None, OP.add
                        )
                        m = sp.tile([128, IC], BF16, tag="m", bufs=3)
                        nc.vector.tensor_tensor(m[:], ea[:], tt[:], OP.max)
                        nc.vector.tensor_tensor(
                            e[:, jb, :], m[:], bslice.bitcast(BF16), OP.mult
                        )
                for jd, jb in enumerate(d_jbs):
                    nc.scalar.activation(
                        e[:, jb, :], lrt[:, jd, :], AF.Exp, bias=0.0, scale=1.0
                    )

                for i128 in range(IC // 128):
                    acc = psp.tile([128, D + 1], F32, tag="acc")
                    for jb in range(NB):
                        nc.tensor.matmul(
                            acc[:],
                            e[:, jb, i128 * 128 : (i128 + 1) * 128],
                            fb[:, jb, 0 : D + 1],
                            start=(jb == 0),
                            stop=(jb == NB - 1),
                        )
                    rcp = sp.tile([128, 1], F32, tag="rcp")
                    nc.vector.reciprocal(rcp[:], acc[:, D : D + 1])
                    nc.vector.scalar_tensor_tensor(
                        obuf[:, i128, :], acc[:, 0:D], rcp[:, 0:1],
                        dnb[:, i128, :], OP.mult, OP.add,
                    )
                nc.sync.dma_start(out_r[:, ic * 4 : (ic + 1) * 4, :], obuf[:])

    _legalize_waits(nc)
    _nc_cache[key] = nc
    return nc


def make_in_maps(data, bias1, W1, W2, b2, bias2):
    """Host-side sharding / layout prep. Core c = 2*b + h."""
    data = np.asarray(data, dtype=np.float32)
    bias1 = np.asarray(bias1, dtype=np.float32)
    W1 = np.asarray(W1, dtype=np.float32)
    W2 = np.asarray(W2, dtype=np.float32)
    b2 = np.asarray(b2, dtype=np.float32)
    bias2 = np.asarray(bias2, dtype=np.float32)

    bf = ml_dtypes.bfloat16
    # augmented weights (see build_nc)
    w1t = np.zeros((D, D + 1), dtype=np.float32)
    w1t[:, 0:D] = W1.T
    w1t[:, D] = (W1.astype(np.float64).T @ W2.astype(np.float64)).astype(np.float32)
    w1tb = w1t.astype(bf)
    b2rep = np.full((128, 1), 2.0 * b2[0], dtype=np.float32)
    ident = np.eye(128, dtype=np.float32)

    b1T = bias1.T  # [j, i]
    d_tiles = [jb for jb in range(NB) if is_d_tile(jb)]

    in_maps = []
    for c in range(NCORES):
        b, h = divmod(c, 2)
        rows = slice(h * R, (h + 1) * R)
        dT = data[b].T  # [D, N]
        if h == 1:
            dT = np.concatenate([dT[:, R:], dT[:, :R]], axis=1)
            eb = np.concatenate([b1T[R:, rows], b1T[:R, rows]], axis=0)
        else:
            eb = b1T[:, rows]
        # per-(ic, jb) blocks: raw fp16 bits for D tiles, exp bf16 for A
        b1t = np.empty((NIC, 4, 128, 8, IC), dtype=np.uint16)
        for jb in range(NB):
            g, q = divmod(jb, 8)
            blk = eb[jb * 128 : (jb + 1) * 128, :]  # [128, R]
            if jb in d_tiles:
                bits = blk.astype(np.float16).view(np.uint16)
            else:
                bits = np.exp(blk).astype(bf).view(np.uint16)
            for ic in range(NIC):
                b1t[ic, g, :, q, :] = bits[:, ic * IC : (ic + 1) * IC]
        in_maps.append(
            {
                "dataT": np.ascontiguousarray(dT.astype(bf)),
                "datan": np.ascontiguousarray(data[b, rows] + bias2[None, :]),
                "w1t": w1tb,
                "b2rep": b2rep,
                "b1t": b1t,
                "ident": ident,
            }
        )
    return in_maps


def assemble(results):
    out = np.empty((B, N, D), dtype=np.float32)
    for c in range(NCORES):
        b, h = divmod(c, 2)
        out[b, h * R : (h + 1) * R, :] = results[c]["out"]
    return out


def kernel(data, bias1, W1, W2, b2, bias2):
    nc = build_nc()
    in_maps = make_in_maps(data, bias1, W1, W2, b2, bias2)
    res = run_bass_kernel_spmd(nc, in_maps, core_ids=list(range(NCORES)))
    return assemble(res.results)


# revision 7
# speedup vs baseline: 1.7119x; 1.5560x over previous
"""GAT head kernel for Trainium2, 8 SPMD NeuronCores (v3).

Reference (B=4, N=4096, D=256):
    feats  = data @ W1.T                          [B,N,D]
    f1     = feats @ W2 + b2                      [B,N]
    logits = f1[:,:,None] + f1[:,None,:]          [B,N,N]
    coefs  = softmax(leaky_relu(logits) + bias1, axis=-1)
    out    = coefs @ feats + bias2 + data

Core c = 2*b + h owns batch b, row half h (R=2048 rows i), all N j's.
E[j,i] = exp(leaky_relu(f1_i+f1_j) + bias1[i,j]) is built in [j(part), i]
tiles (the lhsT the PE wants); a ones column in the matmul rhs yields the
softmax denominator; bias2 + the data residual are host-folded into `datan`.

j-blocks are RENUMBERED into slots: slots 0..ND-1 run pipeline D (exact
leaky, ACT-heavy), slots ND..31 pipeline A (rank-1 exp trick, DVE-heavy).
Contiguous slots let wide (FD 1024-2048) DVE/ACT ops amortize the
per-instruction overhead (DVE 58cyc, ACT 224cyc "errata").

  D:  lr  = Lrelu(f1bc + f1_j + 2b2)   ACT, alpha=.01, fp16, FD=1024 (ic pair)
      z   = lr + bias1_raw(fp16)       DVE TT in-place, 4 slots at once
      e   = exp(z) -> bf16             ACT, 4 slots at once (strided in)
  A:  ea  = abc * a_j                  DVE TS (abc = exp(f1bc), a_j dev-side)
      t   = tbc + s_j                  DVE TS (tbc = 1 + .01 f1bc)
      m   = max(ea, t)                 DVE TT in-place, slot pair
      e   = m * exp(bias1) bf16        DVE TT, slot pair

ACT table only reloads at Lrelu-batch/Exp-batch boundaries (2 per ic pair).
Host uploads per-slot either raw bias1 fp16 bits (D) or exp(bias1) bf16
bits (A) in one uint16 tensor laid out [ic, g, p, q, i] -> 8KB/partition
contiguous DMA lines.
"""

import sys

sys.path.insert(0, "/opt/trn_rl_repo")

import numpy as np
import ml_dtypes

import concourse.bass as bass
import concourse.mybir as mybir
from concourse.tile import TileContext
from concourse.bass_utils import run_bass_kernel_spmd

# ---------------------------------------------------------------- config
B, N, D = 4, 4096, 256
NCORES = 8
R = N * B // NCORES          # rows per core = 2048
NB = N // 128                # j blocks = 32
IC = 512                     # i-chunk width
NIC = R // IC                # i chunks per core = 4
HB = R // 128                # 16: i-blocks of 128 per core

F32 = mybir.dt.float32
BF16 = mybir.dt.bfloat16
F16 = mybir.dt.float16
U16 = mybir.dt.uint16

# knobs
ND = 20                      # slots 0..ND-1 -> pipeline D; rest pipeline A
ZW = 4                       # D slots per z-add / exp op
AW = 2                       # A slots per max / mult op

_nc_cache = {}


def slot_of_jb():
    """jb -> slot: D-tiles (Bresenham-spread over jb) to slots 0..ND-1."""
    d_jbs = [jb for jb in range(NB)
             if ((jb + 1) * ND) // NB > (jb * ND) // NB]
    a_jbs = [jb for jb in range(NB) if jb not in d_jbs]
    slot = {}
    for s, jb in enumerate(d_jbs + a_jbs):
        slot[jb] = s
    return slot, d_jbs, a_jbs


def _legalize_waits(nc, max_inst_waits=1, max_ev_waits=2):
    """Hoist excess sync waits into EventSemaphores on the same engine."""
    counter = 0
    for fn in nc.m.functions:
        for bb in fn.blocks:
            out = []
            changed = False
            for ins in bb.instructions:
                si = ins.sync_info
                waits = list(si.on_wait) if si and si.on_wait else []
                limit = (
                    max_ev_waits
                    if isinstance(ins, mybir.InstEventSemaphore)
                    else max_inst_waits
                )
                if len(waits) > limit:
                    extra, keep = waits[:-limit], waits[-limit:]
                    while extra:
                        chunk, extra = extra[:max_ev_waits], extra[max_ev_waits:]
                        counter += 1
                        ev = mybir.InstEventSemaphore(
                            name=f"waitsplit_{counter}", engine=ins.engine
                        )
                        ev.sync_info = mybir.SyncInfo(on_wait=chunk, on_update=[])
                        out.append(ev)
                        changed = True
                    ins.sync_info = mybir.SyncInfo(
                        on_wait=keep,
                        on_update=list(si.on_update) if si.on_update else [],
                    )
                out.append(ins)
            if changed:
                bb.instructions = out
    return nc


def build_nc():
    key = (ND, ZW, AW, IC)
    if key in _nc_cache:
        return _nc_cache[key]

    nc = bass.Bass()
    AF = mybir.ActivationFunctionType
    OP = mybir.AluOpType

    slot, d_jbs, a_jbs = slot_of_jb()
    NA = NB - ND

    dataT_d = nc.dram_tensor("dataT", [D, N], BF16, kind="ExternalInput")
    datan_d = nc.dram_tensor("datan", [R, D], F32, kind="ExternalInput")
    w1t_d = nc.dram_tensor("w1t", [D, D + 1], BF16, kind="ExternalInput")
    b2rep_d = nc.dram_tensor("b2rep", [128, 1], F32, kind="ExternalInput")
    b1t_d = nc.dram_tensor("b1t", [NIC, 4, 128, 8, IC], U16, kind="ExternalInput")
    ident_d = nc.dram_tensor("ident", [128, 128], F32, kind="ExternalInput")
    out_d = nc.dram_tensor("out", [R, D], F32, kind="ExternalOutput")

    with TileContext(nc) as tc:
        with (
            tc.tile_pool(name="persist", bufs=1) as pp,
            tc.tile_pool(name="epool", bufs=2) as ep,
            tc.tile_pool(name="stream", bufs=2) as sp,
            tc.tile_pool(name="psum", bufs=3, space="PSUM") as psp,
            tc.tile_pool(name="psfeat", bufs=2, space="PSUM") as psf,
        ):
            # ---------------- phase 0: feats, f1, broadcast rows ----------
            # w1t host-augmented to [256, 257]: col 256 = W1.T @ W2 so the
            # feats matmul also emits raw f1 in psum col 256.
            w1t_lo = pp.tile([128, D + 1], BF16, tag="w1lo")
            w1t_hi = pp.tile([128, D + 1], BF16, tag="w1hi")
            nc.sync.dma_start(w1t_lo[:], w1t_d[0:128, :])
            nc.sync.dma_start(w1t_hi[:], w1t_d[128:256, :])
            b2rep = pp.tile([128, 1], F32, tag="b2rep")
            nc.sync.dma_start(b2rep[:], b2rep_d[:])
            b2s = pp.tile([128, 1], F32, tag="b2s")
            nc.vector.tensor_scalar_mul(b2s[:], b2rep[:], 0.01)
            ident = pp.tile([128, 128], F32, tag="ident")
            nc.sync.dma_start(ident[:], ident_d[:])
            ones128 = pp.tile([128, 128], F32, tag="ones128")
            nc.vector.memset(ones128[:], 1.0)

            # feats bf16 (indexed by SLOT) with ones col at [:, :, 256]
            fb = pp.tile([128, NB, D + 2], BF16, tag="fb")
            nc.vector.memset(fb[:, :, D : D + 1], 1.0)
            f1 = pp.tile([128, NB], F32, tag="f1")        # raw f1, by jb
            f1b2 = pp.tile([128, NB], F32, tag="f1b2")    # f1 + 2*b2
            s001 = pp.tile([128, NB], F32, tag="s001")    # .01 f1 + .02 b2
            aj = pp.tile([128, NB], F32, tag="aj")        # exp(f1 + 2 b2)
            f1bc = pp.tile([128, R], F32, tag="f1bc")

            DCH = 1024  # dataT streamed in 128-col-x-8-jb chunks
            dT0c = dT1c = None
            for jb in range(NB):
                if jb % (DCH // 128) == 0:
                    csl = slice(jb * 128, jb * 128 + DCH)
                    dT0c = sp.tile([128, DCH], BF16, tag="dT0c", bufs=2)
                    nc.sync.dma_start(dT0c[:], dataT_d[0:128, csl])
                    dT1c = sp.tile([128, DCH], BF16, tag="dT1c", bufs=2)
                    nc.sync.dma_start(dT1c[:], dataT_d[128:256, csl])
                jsl = slice((jb % (DCH // 128)) * 128, (jb % (DCH // 128) + 1) * 128)
                ps = psf.tile([128, D + 1], F32, tag="featps")
                nc.tensor.matmul(ps[:], dT0c[:, jsl], w1t_lo[:], start=True, stop=False)
                nc.tensor.matmul(ps[:], dT1c[:, jsl], w1t_hi[:], start=False, stop=True)
                nc.vector.tensor_copy(fb[:, slot[jb], 0:D], ps[:, 0:D])
                nc.vector.tensor_copy(f1[:, jb : jb + 1], ps[:, D : D + 1])
                if jb < HB:
                    # own-row broadcast: replicate f1 col jb along free,
                    # PE-transpose into f1bc block (f1 of row i, all parts).
                    colt = sp.tile([128, 128], F32, tag="colt", bufs=3)
                    nc.vector.tensor_scalar_mul(
                        colt[:], ones128[:], f1[:, jb : jb + 1]
                    )
                    psb = psf.tile([128, 128], F32, tag="psb")
                    nc.tensor.transpose(psb[:], colt[:], ident[:])
                    nc.vector.tensor_copy(
                        f1bc[:, jb * 128 : (jb + 1) * 128], psb[:]
                    )
                if jb == HB - 1 or jb == NB - 1:
                    hsl = slice(0, HB) if jb == HB - 1 else slice(HB, NB)
                    nc.vector.tensor_scalar(
                        f1b2[:, hsl], f1[:, hsl], b2rep[:, 0:1], None, OP.add
                    )
                    nc.vector.tensor_scalar(
                        s001[:, hsl], f1[:, hsl], 0.01, b2s[:, 0:1],
                        OP.mult, OP.add,
                    )
                    nc.scalar.activation(
                        aj[:, hsl], f1b2[:, hsl], AF.Exp, bias=0.0, scale=1.0
                    )

            # ---------------- phase 1: E tiles + matmul ----------
            datan_r = datan_d.rearrange("(rb p) o -> p rb o", p=128)
            out_r = out_d.rearrange("(rb p) o -> p rb o", p=128)
            # matmul accumulation order: A slots first (their e arrives
            # early from DVE), D slots last (e lands in the final Exp batch)
            mm_slots = list(range(ND, NB)) + list(range(ND))
            for icp in range(NIC // 2):
                # Lrelu batch for the ic PAIR: one op spans both i-chunks
                psl = slice(icp * 2 * IC, (icp + 1) * 2 * IC)
                lrt = sp.tile([128, ND, 2 * IC], F16, tag="lrt", bufs=1)
                for s in range(ND):
                    jb = d_jbs[s]
                    nc.scalar.activation(
                        lrt[:, s, :], f1bc[:, psl], AF.Lrelu,
                        bias=f1b2[:, jb : jb + 1], scale=1.0, alpha=0.01,
                    )
                for ici in range(2):
                    ic = icp * 2 + ici
                    icsl = slice(ic * IC, (ic + 1) * IC)
                    e = ep.tile([128, NB, IC], BF16, tag="e")
                    b1g = [None] * 4
                    for g in range(4):
                        b1g[g] = sp.tile([128, 8, IC], U16, bufs=2,
                                         name=f"b1g{g}", tag=f"b1g{g % 2}")
                        nc.sync.dma_start(b1g[g][:], b1t_d[ic, g])
                    dnb = sp.tile([128, 4, D], F32, tag="dnb", bufs=2)
                    nc.sync.dma_start(dnb[:], datan_r[:, ic * 4 : (ic + 1) * 4, :])
                    obuf = sp.tile([128, 4, D], F32, tag="obuf", bufs=2)

                    # per-ic broadcast tiles for pipeline A
                    abc = sp.tile([128, IC], BF16, tag="abc", bufs=2)
                    nc.scalar.activation(abc[:], f1bc[:, icsl], AF.Exp,
                                         bias=0.0, scale=1.0)
                    tbc = sp.tile([128, IC], BF16, tag="tbc", bufs=2)
                    nc.vector.tensor_scalar(tbc[:], f1bc[:, icsl], 0.01, 1.0,
                                            OP.mult, OP.add)

                    # D: z = lr + bias1_raw, in-place, ZW slots per op
                    lrs = lrt[:, :, ici * IC : (ici + 1) * IC]
                    for s0 in range(0, ND, ZW):
                        w = min(ZW, ND - s0)
                        g, q = divmod(s0, 8)
                        assert q + w <= 8
                        nc.vector.tensor_tensor(
                            lrs[:, s0 : s0 + w, :],
                            lrs[:, s0 : s0 + w, :],
                            b1g[g][:, q : q + w, :].bitcast(F16),
                            OP.add,
                        )
                    # A: rank-1 exp + affine t + max + bias mult, AW slots/op
                    for k0 in range(0, NA, AW):
                        w = min(AW, NA - k0)
                        s0 = ND + k0
                        ea = sp.tile([128, AW, IC], BF16, tag="ea", bufs=2)
                        tt = sp.tile([128, AW, IC], BF16, tag="tt", bufs=2)
                        for k in range(w):
                            jb = a_jbs[k0 + k]
                            nc.vector.tensor_scalar(
                                ea[:, k, :], abc[:], aj[:, jb : jb + 1],
                                None, OP.mult,
                            )
                            nc.vector.tensor_scalar(
                                tt[:, k, :], tbc[:], s001[:, jb : jb + 1],
                                None, OP.add,
                            )
                        nc.vector.tensor_tensor(
                            ea[:, 0:w, :], ea[:, 0:w, :], tt[:, 0:w, :], OP.max
                        )
                        g, q = divmod(s0, 8)
                        assert q + w <= 8
                        nc.vector.tensor_tensor(
                            e[:, s0 : s0 + w, :], ea[:, 0:w, :],
                            b1g[g][:, q : q + w, :].bitcast(BF16), OP.mult,
                        )
                    # D: e = exp(z), ZW slots per op (strided read from lrt)
                    for s0 in range(0, ND, ZW):
                        w = min(ZW, ND - s0)
                        nc.scalar.activation(
                            e[:, s0 : s0 + w, :], lrs[:, s0 : s0 + w, :],
                            AF.Exp, bias=0.0, scale=1.0,
                        )

                    for i128 in range(IC // 128):
                        acc = psp.tile([128, D + 1], F32, tag="acc")
                        for mi, s in enumerate(mm_slots):
                            nc.tensor.matmul(
                                acc[:],
                                e[:, s, i128 * 128 : (i128 + 1) * 128],
                                fb[:, s, 0 : D + 1],
                                start=(mi == 0),
                                stop=(mi == NB - 1),
                            )
                        rcp = sp.tile([128, 1], F32, tag="rcp", bufs=3)
                        nc.vector.reciprocal(rcp[:], acc[:, D : D + 1])
                        nc.vector.scalar_tensor_tensor(
                            obuf[:, i128, :], acc[:, 0:D], rcp[:, 0:1],
                            dnb[:, i128, :], OP.mult, OP.add,
                        )
                    nc.sync.dma_start(out_r[:, ic * 4 : (ic + 1) * 4, :], obuf[:])

    _legalize_waits(nc)
    _nc_cache[key] = nc
    return nc


def make_in_maps(data, bias1, W1, W2, b2, bias2):
    """Host-side sharding / layout prep. Core c = 2*b + h."""
    data = np.asarray(data, dtype=np.float32)
    bias1 = np.asarray(bias1, dtype=np.float32)
    W1 = np.asarray(W1, dtype=np.float32)
    W2 = np.asarray(W2, dtype=np.float32)
    b2 = np.asarray(b2, dtype=np.float32)
    bias2 = np.asarray(bias2, dtype=np.float32)

    bf = ml_dtypes.bfloat16
    slot, d_jbs, a_jbs = slot_of_jb()
    # augmented weights (see build_nc)
    w1t = np.zeros((D, D + 1), dtype=np.float32)
    w1t[:, 0:D] = W1.T
    w1t[:, D] = (W1.astype(np.float64).T @ W2.astype(np.float64)).astype(np.float32)
    w1tb = w1t.astype(bf)
    b2rep = np.full((128, 1), 2.0 * b2[0], dtype=np.float32)
    ident = np.eye(128, dtype=np.float32)

    b1T = bias1.T  # [j, i]

    in_maps = []
    for c in range(NCORES):
        b, h = divmod(c, 2)
        rows = slice(h * R, (h + 1) * R)
        dT = data[b].T  # [D, N]
        if h == 1:
            dT = np.concatenate([dT[:, R:], dT[:, :R]], axis=1)
            eb = np.concatenate([b1T[R:, rows], b1T[:R, rows]], axis=0)
        else:
            eb = b1T[:, rows]
        # per-(ic, slot) blocks: raw fp16 bits for D slots, exp bf16 for A
        b1t = np.empty((NIC, 4, 128, 8, IC), dtype=np.uint16)
        for jb in range(NB):
            s = slot[jb]
            g, q = divmod(s, 8)
            blk = eb[jb * 128 : (jb + 1) * 128, :]  # [128, R]
            if s < len(d_jbs):
                bits = blk.astype(np.float16).view(np.uint16)
            else:
                bits = np.exp(blk).astype(bf).view(np.uint16)
            for ic in range(NIC):
                b1t[ic, g, :, q, :] = bits[:, ic * IC : (ic + 1) * IC]
        in_maps.append(
            {
                "dataT": np.ascontiguousarray(dT.astype(bf)),
                "datan": np.ascontiguousarray(data[b, rows] + bias2[None, :]),
                "w1t": w1tb,
                "b2rep": b2rep,
                "b1t": b1t,
                "ident": ident,
            }
        )
    return in_maps


def assemble(results):
    out = np.empty((B, N, D), dtype=np.float32)
    for c in range(NCORES):
        b, h = divmod(c, 2)
        out[b, h * R : (h + 1) * R, :] = results[c]["out"]
    return out


def kernel(data, bias1, W1, W2, b2, bias2):
    nc = build_nc()
    in_maps = make_in_maps(data, bias1, W1, W2, b2, bias2)
    res = run_bass_kernel_spmd(nc, in_maps, core_ids=list(range(NCORES)))
    return assemble(res.results)
